# revision 28
# baseline (speedup 1.0000x reference)
"""Trainium2 Bass kernel for AttentionSequencePoolingLayer (DIN-style attention).

Reference computation (per batch b):
    att_in = concat([q, k, q-k, q*k], -1)            (T, 4E)
    h1 = relu(att_in @ W1 + b1)                      (T, 80)
    h2 = relu(h1 @ W2 + b2)                          (T, 40)
    s  = h2 @ W3 + b3                                (T, 1)
    out = (mask * s).T @ k                           (1, E)

Algebraic restructuring (cuts layer-1 FLOPs 4x): q is constant over T, so
with W1 = [W1a; W1b; W1c; W1d] (blocks of E rows):
    h1.T = relu( (W1b-W1c).T @ k.T  +  W1d.T @ (q (.) k.T)  +  C[:, b] )
    C = (W1a+W1c).T @ Q.T + b1      (one matmul for all batches)
The per-batch bias C[:, b] is injected into the same PSUM accumulation with a
K=2 "selector" matmul (lhsT = two C columns, rhs = 0/1 selector rows), so the
relu evacuation needs no per-batch bias and is a single wide op.

Scores are produced directly on T-partitions by small transposed matmuls
(lhsT = h2.T column slices, rhs = padded W3), masked during evacuation with
pre-transposed masks (b3 rides in as b3*mask), then pooling accumulates
poolT[:, b] = keys_nat.T @ masked_scores as single-column matmuls (bf16
weights -> fast weight load). fp32r everywhere else (N >= 256 keeps full PE
rate; N=1 is ISA-illegal for fp32r so score/pool columns are padded to N=2).

Sharding: pure data parallel, batch dim split across 8 NeuronCores
(256 batches per core), 16-batch supergroups (two ~1MB strided DMAs each),
2-batch matmul groups (moving dim N=400).
"""

from contextlib import ExitStack

import numpy as np

import concourse.bass as bass
import concourse.bacc as bacc
import concourse.tile as tile
from concourse import mybir
from concourse.bass_utils import run_bass_kernel_spmd
from concourse.masks import make_identity

B, T, E = 2048, 200, 128
H1, H2 = 80, 40
N_CORES = 8
B_CORE = B // N_CORES   # 256
SG = 16                 # batches per supergroup (keys-DMA granularity)
GRP = 2                 # batches per matmul group (N = GRP*T = 400)
TA, TB = 128, T - 128   # key-row split across partitions

F32 = mybir.dt.float32
F32R = mybir.dt.float32r
BF16 = mybir.dt.bfloat16
U8 = mybir.dt.uint8
AF = mybir.ActivationFunctionType
OP = mybir.AluOpType


def build(b_core=B_CORE):
    nc = bacc.Bacc("TRN2", target_bir_lowering=False, debug=False,
                   num_devices=N_CORES)
    q_d = nc.dram_tensor("query", [b_core, 1, E], F32, kind="ExternalInput")
    k_d = nc.dram_tensor("keys", [b_core, T, E], F32R, kind="ExternalInput")
    m_d = nc.dram_tensor("key_masks", [b_core, 1, T], U8, kind="ExternalInput")
    w1_d = nc.dram_tensor("W1", [4 * E, H1], F32, kind="ExternalInput")
    b1_d = nc.dram_tensor("b1", [H1], F32, kind="ExternalInput")
    w2_d = nc.dram_tensor("W2", [H1, H2], F32, kind="ExternalInput")
    b2_d = nc.dram_tensor("b2", [H2], F32, kind="ExternalInput")
    w3_d = nc.dram_tensor("W3", [H2, 1], F32, kind="ExternalInput")
    b3_d = nc.dram_tensor("b3", [1], F32, kind="ExternalInput")
    out_d = nc.dram_tensor("out", [b_core, 1, E], F32, kind="ExternalOutput")

    with tile.TileContext(nc) as tc:
        _body(tc, nc, q_d, k_d, m_d, w1_d, b1_d, w2_d, b2_d, w3_d, b3_d,
              out_d, b_core)
    nc.compile()
    return nc


def _body(tc, nc, q_d, k_d, m_d, w1_d, b1_d, w2_d, b2_d, w3_d, b3_d, out_d,
          b_core):
    ctx = ExitStack()
    n_g = b_core // GRP
    with ctx:
        consts = ctx.enter_context(tc.tile_pool(name="consts", bufs=1))
        prep = ctx.enter_context(tc.tile_pool(name="prep", bufs=2))
        # shared psum bank: preamble/epilogue scratch + supergroup columns
        pp_ps = ctx.enter_context(
            tc.tile_pool(name="pp_ps", bufs=1, space="PSUM"))

        ident = consts.tile([128, 128], F32)
        make_identity(nc, ident)
        ident_r = consts.tile([128, 128], F32R)
        nc.vector.tensor_copy(out=ident_r, in_=ident)

        # ---- weights ----
        w1s = consts.tile([128, 4, H1], F32)
        nc.sync.dma_start(out=w1s, in_=w1_d.rearrange("(f p) c -> p f c", p=128))
        w1bc = consts.tile([128, H1], F32R)
        nc.vector.tensor_tensor(out=w1bc, in0=w1s[:, 1, :], in1=w1s[:, 2, :],
                                op=OP.subtract)
        w1ac = consts.tile([128, H1], F32R)
        nc.vector.tensor_tensor(out=w1ac, in0=w1s[:, 0, :], in1=w1s[:, 2, :],
                                op=OP.add)
        w1db = consts.tile([128, H1], BF16)
        nc.vector.tensor_copy(out=w1db, in_=w1s[:, 3, :])
        w2f = consts.tile([H1, H2], F32)
        nc.sync.dma_start(out=w2f, in_=w2_d.ap())
        w2 = consts.tile([H1, H2], F32R)
        nc.vector.tensor_copy(out=w2, in_=w2f)
        w3f = consts.tile([H2, 1], F32)
        nc.sync.dma_start(out=w3f, in_=w3_d.ap())
        w3pf = consts.tile([H2, 2], F32)
        nc.vector.memset(w3pf, 0.0)
        nc.vector.tensor_copy(out=w3pf[:, 0:1], in_=w3f)
        w3pb = consts.tile([H2, 2], BF16)
        nc.vector.tensor_copy(out=w3pb, in_=w3pf)
        b1 = consts.tile([H1, 1], F32)
        nc.sync.dma_start(
            out=b1, in_=bass.AP(tensor=b1_d.ap().tensor, offset=0,
                                ap=[[1, H1], [1, 1]]))
        b2 = consts.tile([H2, 1], F32)
        nc.sync.dma_start(
            out=b2, in_=bass.AP(tensor=b2_d.ap().tensor, offset=0,
                                ap=[[1, H2], [1, 1]]))
        b3bc = consts.tile([128, 1], F32)
        nc.sync.dma_start(
            out=b3bc, in_=bass.AP(tensor=b3_d.ap().tensor, offset=0,
                                  ap=[[0, 128], [1, 1]]))
        zerob = consts.tile([128, 2 * SG], BF16)
        nc.vector.memset(zerob, 0.0)

        # ---- Q.T (E on partitions, batch on free) ----
        qt = consts.tile([128, b_core], F32R)
        q_flat = q_d.rearrange("b 1 e -> b e")
        for i in range(0, b_core, 128):
            cnt = min(128, b_core - i)
            qnat = prep.tile([128, E], F32, tag="qnat")
            nc.sync.dma_start(out=qnat[:cnt, :], in_=q_flat[i:i + cnt, :])
            qps = pp_ps.tile([128, 256], F32, tag="pps")
            nc.tensor.transpose(qps[:, :cnt], qnat[:cnt, :], ident[:cnt, :cnt])
            nc.vector.tensor_copy(out=qt[:, i:i + cnt], in_=qps[:, :cnt])

        # ---- C = (W1a+W1c).T @ Q.T + b1, repacked for K=2 selector matmuls:
        # ct_all[j, g, :] = C[:, GRP*g + j]
        cps = pp_ps.tile([128, 256], F32, tag="pps")
        nc.tensor.matmul(cps[:H1, :b_core], lhsT=w1ac, rhs=qt,
                         start=True, stop=True)
        csb = consts.tile([H1, b_core], F32)
        nc.scalar.activation(out=csb, in_=cps[:H1, :b_core], func=AF.Identity,
                             bias=b1)

        # ---- masks, transposed to (t, batch); plus b3 * mask ----
        mt0 = consts.tile([TA, b_core], F32)
        mt1 = consts.tile([TB, b_core], F32)
        m_flat = m_d.rearrange("b 1 t -> b t")
        for i in range(0, b_core, 128):
            cnt = min(128, b_core - i)
            mu8 = prep.tile([128, T], U8, tag="mu8")
            nc.sync.dma_start(out=mu8[:cnt, :], in_=m_flat[i:i + cnt, :])
            mf = prep.tile([128, T], F32, tag="mf")
            nc.vector.tensor_copy(out=mf[:cnt, :], in_=mu8[:cnt, :])
            mp0 = pp_ps.tile([128, 256], F32, tag="pps")
            nc.tensor.transpose(mp0[:TA, :cnt], mf[:cnt, 0:TA],
                                ident[:cnt, :cnt])
            nc.vector.tensor_copy(out=mt0[:, i:i + cnt], in_=mp0[:TA, :cnt])
            mp1 = pp_ps.tile([128, 256], F32, tag="pps")
            nc.tensor.transpose(mp1[:TB, :cnt], mf[:cnt, TA:T],
                                ident[:cnt, :cnt])
            nc.vector.tensor_copy(out=mt1[:, i:i + cnt], in_=mp1[:TB, :cnt])
        b3m0 = consts.tile([TA, b_core], F32)
        nc.vector.tensor_scalar_mul(b3m0, mt0, b3bc[0:TA, :])
        b3m1 = consts.tile([TB, b_core], F32)
        nc.vector.tensor_scalar_mul(b3m1, mt1, b3bc[0:TB, :])

        # pooled output, transposed: (E, batch)
        poolt_sb = consts.tile([128, b_core], F32)

        # ---- main pipeline pools ----
        kstA = ctx.enter_context(tc.tile_pool(name="kstA", bufs=2))
        kstB = ctx.enter_context(tc.tile_pool(name="kstB", bufs=2))
        ktp = ctx.enter_context(tc.tile_pool(name="ktp", bufs=6))
        work = ctx.enter_context(tc.tile_pool(name="work", bufs=6))
        stp = ctx.enter_context(tc.tile_pool(name="stp", bufs=2))
        pk_ps = ctx.enter_context(tc.tile_pool(name="pk_ps", bufs=3, space="PSUM"))
        h1_ps = ctx.enter_context(tc.tile_pool(name="h1_ps", bufs=2, space="PSUM"))
        h2_ps = ctx.enter_context(tc.tile_pool(name="h2_ps", bufs=1, space="PSUM"))
        sm_ps = ctx.enter_context(tc.tile_pool(name="sm_ps", bufs=1, space="PSUM"))

        n_sg = (b_core + SG - 1) // SG
        NCOL = GRP * T  # 400

        for sg in range(n_sg):
            b0 = sg * SG
            nb = min(SG, b_core - b0)
            # big strided loads: natural keys, t on partitions, batch on free
            tA = kstA.tile([TA, SG, E], F32R, tag="tA")
            nc.sync.dma_start(
                out=tA[:, :nb, :],
                in_=k_d[b0:b0 + nb, 0:TA, :].rearrange("b t e -> t b e"))
            tB = kstB.tile([TB, SG, E], F32R, tag="tB")
            nc.sync.dma_start(
                out=tB[:, :nb, :],
                in_=k_d[b0:b0 + nb, TA:T, :].rearrange("b t e -> t b e"))
            # bf16 copies for the pooling weight loads (fast weight load)
            tAb = kstA.tile([TA, SG, E], BF16, tag="tAb")
            nc.vector.tensor_copy(out=tAb[:, :nb, :],
                                  in_=tA[:, :nb, :].bitcast(F32))
            tBb = kstB.tile([TB, SG, E], BF16, tag="tBb")
            nc.vector.tensor_copy(out=tBb[:, :nb, :],
                                  in_=tB[:, :nb, :].bitcast(F32))

            # per-supergroup psum bank: score columns and pooled columns share
            # one bank; every matmul into it is atomic (start+stop) over
            # disjoint columns, so bank-wide has_written clears are harmless.
            smbig = sm_ps.tile([128, 8 * SG], F32, tag="smbig")
            stA_ps = smbig[:, 0:2 * SG]
            stB_ps = smbig[0:TB, 2 * SG:4 * SG]
            plTA_ps = smbig[:, 4 * SG:6 * SG]
            plTB_ps = smbig[:, 6 * SG:8 * SG]

            # groups are emitted pairwise, phase by phase, so each
            # cross-engine handoff has a full phase of slack to complete
            # before the consumer issues on its engine. The score minis of
            # the previous pair are emitted between this pair's L1 chain and
            # L2 so the PE has work while relu1 runs on ACT.
            def emit_minis(h2_list):
                for lb, h2 in h2_list:
                    for j in range(GRP):
                        c = j * T
                        o = 2 * (lb + j)
                        nc.tensor.matmul(stA_ps[:, o:o + 2],
                                         lhsT=h2[:, c:c + TA], rhs=w3pb,
                                         start=True, stop=True)
                        nc.tensor.matmul(stB_ps[:, o:o + 2],
                                         lhsT=h2[:, c + TA:c + T], rhs=w3pb,
                                         start=True, stop=True)

            pending = []
            for g0 in range(0, nb // GRP, 2):
                pair = [g for g in (g0, g0 + 1) if g < nb // GRP]
                st = {}
                for g in pair:
                    lb = GRP * g
                    ktps = pk_ps.tile([128, NCOL], F32R, tag="ktps")
                    for j in range(GRP):
                        c = j * T
                        nc.tensor.transpose(ktps[:, c:c + TA],
                                            tA[:, lb + j, :], ident_r)
                        nc.tensor.transpose(ktps[:, c + TA:c + T],
                                            tB[:, lb + j, :],
                                            ident_r[:TB, :TB])
                    st[g] = {"ktps": ktps}
                for g in pair:
                    kt = ktp.tile([128, NCOL], F32R, tag="kt")
                    nc.vector.tensor_copy(out=kt, in_=st[g]["ktps"])
                    st[g]["kt"] = kt
                for g in pair:
                    gb = b0 + GRP * g
                    kt = st[g]["kt"]
                    qk = ktp.tile([128, NCOL], BF16, tag="qk")
                    for j in range(GRP):
                        nc.vector.tensor_scalar_mul(
                            qk[:, j * T:(j + 1) * T],
                            kt[:, j * T:(j + 1) * T].bitcast(F32),
                            qt[:, gb + j:gb + j + 1].bitcast(F32))
                    st[g]["qk"] = qk
                for g in pair:
                    gb = b0 + GRP * g
                    h1p = h1_ps.tile([H1, NCOL], F32, tag="h1p")
                    nc.tensor.matmul(h1p, lhsT=w1bc, rhs=st[g]["kt"],
                                     start=True, stop=False)
                    nc.tensor.matmul(h1p, lhsT=w1db, rhs=st[g]["qk"],
                                     start=False, stop=True)
                    st[g]["h1p"] = h1p
                for g in pair:
                    gb = b0 + GRP * g
                    h1 = work.tile([H1, NCOL], F32R, tag="h1")
                    for j in range(GRP):
                        nc.scalar.activation(
                            out=h1[:, j * T:(j + 1) * T],
                            in_=st[g]["h1p"][:, j * T:(j + 1) * T],
                            func=AF.Relu, bias=csb[:, gb + j:gb + j + 1])
                    st[g]["h1"] = h1
                emit_minis(pending)
                pending = []
                for g in pair:
                    h2p = h2_ps.tile([H2, NCOL], F32, tag="h2p")
                    nc.tensor.matmul(h2p, lhsT=w2, rhs=st[g]["h1"],
                                     start=True, stop=True)
                    st[g]["h2p"] = h2p
                for g in pair:
                    h2 = work.tile([H2, NCOL], BF16, tag="h2")
                    nc.scalar.activation(out=h2, in_=st[g]["h2p"],
                                         func=AF.Relu, bias=b2)
                    st[g]["h2"] = h2
                    pending.append((GRP * g, h2))
            emit_minis(pending)
            pending = []

            # masked scores: sT_m = sT * m + b3 * m   (b3m precomputed)
            stA_s = stA_ps.rearrange("p (b two) -> p b two", two=2)[:, :, 0]
            stB_s = stB_ps.rearrange("p (b two) -> p b two", two=2)[:, :, 0]
            stA = stp.tile([TA, 2 * SG], BF16, tag="stA")
            nc.vector.tensor_copy(out=stA, in_=zerob[:TA, :])
            stAv = stA.rearrange("p (b two) -> p b two", two=2)[:, :, 0]
            nc.vector.tensor_tensor(out=stAv[:, :nb], in0=stA_s[:, :nb],
                                    in1=mt0[:, b0:b0 + nb], op=OP.mult)
            nc.vector.tensor_tensor(out=stAv[:, :nb], in0=stAv[:, :nb],
                                    in1=b3m0[:, b0:b0 + nb], op=OP.add)
            stB = stp.tile([TB, 2 * SG], BF16, tag="stB")
            nc.vector.tensor_copy(out=stB, in_=zerob[:TB, :])
            stBv = stB.rearrange("p (b two) -> p b two", two=2)[:, :, 0]
            nc.vector.tensor_tensor(out=stBv[:, :nb], in0=stB_s[:, :nb],
                                    in1=mt1[:, b0:b0 + nb], op=OP.mult)
            nc.vector.tensor_tensor(out=stBv[:, :nb], in0=stBv[:, :nb],
                                    in1=b3m1[:, b0:b0 + nb], op=OP.add)

            # pooling: poolT[:, b] = knat_A.T @ sTm_A + knat_B.T @ sTm_B
            # (halves land in separate psum columns, summed on evacuation)
            for lb in range(nb):
                nc.tensor.matmul(plTA_ps[:, 2 * lb:2 * lb + 2],
                                 lhsT=tAb[:, lb, :],
                                 rhs=stA[:, 2 * lb:2 * lb + 2],
                                 start=True, stop=True)
                nc.tensor.matmul(plTB_ps[:, 2 * lb:2 * lb + 2],
                                 lhsT=tBb[:, lb, :],
                                 rhs=stB[:, 2 * lb:2 * lb + 2],
                                 start=True, stop=True)
            plA_s = plTA_ps.rearrange("p (b two) -> p b two", two=2)[:, :, 0]
            plB_s = plTB_ps.rearrange("p (b two) -> p b two", two=2)[:, :, 0]
            pltmp = stp.tile([128, SG], F32, tag="pltmp")
            nc.vector.tensor_copy(out=pltmp[:, :nb], in_=plA_s[:, :nb])
            nc.vector.tensor_tensor(out=poolt_sb[:, b0:b0 + nb],
                                    in0=plB_s[:, :nb], in1=pltmp[:, :nb],
                                    op=OP.add)

        # ---- final: transpose pooled back to (batch, E) and store ----
        out_flat = out_d.rearrange("b 1 e -> b e")
        for i in range(0, b_core, 128):
            cnt = min(128, b_core - i)
            ops = pp_ps.tile([128, 256], F32, tag="pps")
            nc.tensor.transpose(ops[:cnt, :128], poolt_sb[:, i:i + cnt], ident)
            onat = prep.tile([128, E], F32, tag="onat")
            nc.vector.tensor_copy(out=onat[:cnt, :], in_=ops[:cnt, :128])
            nc.sync.dma_start(out=out_flat[i:i + cnt, :], in_=onat[:cnt, :])


_NC_CACHE = {}


def _get_nc(b_core=B_CORE):
    if b_core not in _NC_CACHE:
        _NC_CACHE[b_core] = build(b_core)
    return _NC_CACHE[b_core]


def kernel(query, keys, key_masks, W1, b1, W2, b2, W3, b3, _trace=False):
    query = np.ascontiguousarray(query, dtype=np.float32)
    keys = np.ascontiguousarray(keys, dtype=np.float32)
    masks_u8 = np.ascontiguousarray(key_masks).view(np.uint8)
    nc = _get_nc()
    in_maps = []
    for c in range(N_CORES):
        sl = slice(c * B_CORE, (c + 1) * B_CORE)
        in_maps.append({
            "query": query[sl],
            "keys": keys[sl],
            "key_masks": masks_u8[sl],
            "W1": np.asarray(W1, dtype=np.float32),
            "b1": np.asarray(b1, dtype=np.float32),
            "W2": np.asarray(W2, dtype=np.float32),
            "b2": np.asarray(b2, dtype=np.float32),
            "W3": np.asarray(W3, dtype=np.float32),
            "b3": np.asarray(b3, dtype=np.float32),
        })
    res = run_bass_kernel_spmd(nc, in_maps, list(range(N_CORES)), trace=_trace)
    out = np.concatenate([res.results[c]["out"] for c in range(N_CORES)], axis=0)
    if _trace:
        kernel.last_exec_time_ns = res.exec_time_ns
        kernel.last_results = res
    return out.astype(np.float32)


kernel.last_exec_time_ns = None
kernel.last_results = None
